# revision 4
# baseline (speedup 1.0000x reference)
"""Trainium2 Bass kernel for a char-GRU model.

Model: emb = embed[x]; gi = emb @ W_ih + b_ih  (precomputable per token)
       GRU scan over S=512 steps (h = (1-z)*n + z*h), then h_seq @ W_out + b_out.
Shapes: B=128, S=512, E=16, H=256, V=256.

Sharding: data-parallel over batch across 8 cores (16 batch elems/core),
GRU weights replicated.

Device-side plan per core (all compute on device):
  Phase 1: fused token table T' = embed @ W_ih + biases  ([256, 768]),
           one-hot(x) built via PE broadcast + DVE compare,
           gi^T = T'^T @ onehot as bf16 hi/lo split matmuls (exact selection),
           streamed to DRAM scratch in transposed layout [6, 128, S, 16].
  Phase 2: sequential scan; per step: gh^T = W_hh^T-tiles @ h^T on PE
           (output [gates, batch] so gate math runs on 128 partitions),
           sigmoid/tanh on ACT, gate arithmetic on DVE. h^T history kept in
           SBUF [128, S, 2, 16].
  Phase 3: y = h_seq @ W_out + b_out with h_seq tiles stationary,
           written straight to the [B_l, S, V] output layout.
"""
import os
import sys

for _p in ("/opt/trn_rl_repo", "/root/.axon_site/_ro/trn_rl_repo"):
    if os.path.isdir(_p) and _p not in sys.path:
        sys.path.insert(0, _p)

import numpy as np

import concourse.bacc as bacc
import concourse.mybir as mybir
import concourse.tile as tile
from concourse import bass_utils

F32 = mybir.dt.float32
BF16 = mybir.dt.bfloat16
I32 = mybir.dt.int32
AF = mybir.ActivationFunctionType

B, S, E, H, V = 128, 512, 16, 256, 256
NCORES = 8
BL = B // NCORES          # 16 batch elems per core
G3 = 3 * H                # 768
NM = G3 // 128            # 6 gate tiles of 128
TC = 32                   # gi streaming chunk (steps)
NTOK = S * BL             # 8192 tokens per core
NJC = NTOK // 512         # 16 onehot column chunks

_CACHE: dict = {}


def _build():
    if "nc" in _CACHE:
        return _CACHE["nc"]
    nc = bacc.Bacc("TRN2", target_bir_lowering=False, debug=False)

    xt_d = nc.dram_tensor("xt", [S, BL], I32, kind="ExternalInput")
    aaug_d = nc.dram_tensor("a_aug", [E + 1, V], F32, kind="ExternalInput")
    baug_d = nc.dram_tensor("b_aug", [E + 1, G3], F32, kind="ExternalInput")
    whh_d = nc.dram_tensor("w_hh", [H, G3], F32, kind="ExternalInput")
    bn_d = nc.dram_tensor("bn", [128, 2 * BL], F32, kind="ExternalInput")
    wout_d = nc.dram_tensor("w_out", [H, V], F32, kind="ExternalInput")
    bout_d = nc.dram_tensor("b_out", [1, V], F32, kind="ExternalInput")
    y_d = nc.dram_tensor("y", [BL, S, V], F32, kind="ExternalOutput")
    gi_d = nc.dram_tensor("gi_scr", [NM, 128, S, BL], F32, kind="Internal")

    with tile.TileContext(nc) as tc:
        with tc.tile_pool(name="consts", bufs=1) as cp:
            whh_sb = cp.tile([128, 2, G3], F32)
            nc.sync.dma_start(whh_sb[:, 0, :], whh_d.ap()[0:128, :])
            nc.sync.dma_start(whh_sb[:, 1, :], whh_d.ap()[128:256, :])
            wout_sb = cp.tile([128, 2, V], F32)
            nc.sync.dma_start(wout_sb[:, 0, :], wout_d.ap()[0:128, :])
            nc.sync.dma_start(wout_sb[:, 1, :], wout_d.ap()[128:256, :])
            bn_sb = cp.tile([128, 2, BL], F32)
            nc.sync.dma_start(bn_sb[:], bn_d.ap().rearrange("p (c b) -> p c b", c=2))
            bout_sb = cp.tile([1, V], F32)
            nc.sync.dma_start(bout_sb[:], bout_d.ap()[:])
            ones = cp.tile([1, 128], F32)
            nc.vector.memset(ones[:], 1.0)
            io_f = cp.tile([128, 2], F32)
            io_i = cp.tile([128, 1], I32)
            nc.gpsimd.iota(io_i[:], pattern=[[0, 1]], base=0, channel_multiplier=1)
            nc.vector.tensor_copy(io_f[:, 0:1], io_i[:])
            nc.vector.tensor_scalar_add(io_f[:, 1:2], io_f[:, 0:1], 128.0)
            # h^T history: [p, s_block(64), chunk(2), s_in_block(8), b]
            # so phase-3 lhsT slices [p, jb, c, :, :] are contiguous 128-col tiles
            hseq = cp.tile([128, S // 8, 2, 8, BL], F32)   # 64KB/partition
            h0 = cp.tile([128, 2, BL], F32)
            nc.vector.memset(h0[:], 0.0)
            # bf16 hi/lo fused token table, persists through phase 1
            thi = cp.tile([128, 2, G3], BF16)
            tlo = cp.tile([128, 2, G3], BF16)

            # ---------------- Phase 1a: token table T' ----------------
            with (
                tc.tile_pool(name="p1a", bufs=1) as p1,
                tc.tile_pool(name="ps1a", bufs=2, space="PSUM") as ps1,
            ):
                aaug_sb = p1.tile([E + 1, V], F32)
                nc.sync.dma_start(aaug_sb[:], aaug_d.ap()[:])
                baug_sb = p1.tile([E + 1, G3], F32)
                nc.sync.dma_start(baug_sb[:], baug_d.ap()[:])
                for vc in range(2):
                    for nh in range(2):
                        tp_ps = ps1.tile([128, 384], F32, tag="tp")
                        nc.tensor.matmul(
                            tp_ps[:],
                            aaug_sb[:, vc * 128:(vc + 1) * 128],
                            baug_sb[:, nh * 384:(nh + 1) * 384],
                            start=True, stop=True,
                        )
                        sl = slice(nh * 384, (nh + 1) * 384)
                        nc.vector.tensor_copy(thi[:, vc, sl], tp_ps[:])
                        nc.vector.tensor_sub(tlo[:, vc, sl], tp_ps[:], thi[:, vc, sl])

            # ---------------- Phase 1b/1c: onehot + gi ----------------
            with (
                tc.tile_pool(name="p1b", bufs=1) as pb,
                tc.tile_pool(name="p1st", bufs=3) as pst,
                tc.tile_pool(name="ps1b", bufs=2, space="PSUM") as psb,
            ):
                xi = pb.tile([1, NTOK], I32)
                nc.gpsimd.dma_start(xi[:], xt_d.ap().rearrange("(o s) b -> o (s b)", o=1))
                xf = pb.tile([1, NTOK], F32)
                nc.vector.tensor_copy(xf[:], xi[:])
                oh = pb.tile([128, 2, NTOK], BF16)   # 32KB/partition
                for jc in range(NJC):
                    sl = slice(jc * 512, (jc + 1) * 512)
                    xb_ps = psb.tile([128, 512], F32, tag="xb")
                    nc.tensor.matmul(xb_ps[:], ones[0:1, :], xf[0:1, sl],
                                     start=True, stop=True)
                    for c in range(2):
                        nc.vector.tensor_scalar(
                            oh[:, c, sl], xb_ps[:], io_f[:, c:c + 1], None,
                            op0=mybir.AluOpType.is_equal,
                        )
                # gi^T = T'^T @ onehot, bf16 hi/lo accumulated in fp32 PSUM
                for m in range(6):
                    msl = slice(m * 128, (m + 1) * 128)
                    for jc in range(NJC):
                        jsl = slice(jc * 512, (jc + 1) * 512)
                        g_ps = psb.tile([128, 512], F32, tag="gp")
                        first = True
                        for tt in (thi, tlo):
                            for k in range(2):
                                nc.tensor.matmul(
                                    g_ps[:], tt[:, k, msl], oh[:, k, jsl],
                                    start=first, stop=(tt is tlo and k == 1),
                                )
                                first = False
                        gst = pst.tile([128, 512], F32, tag="gst")
                        nc.vector.tensor_copy(gst[:], g_ps[:])
                        nc.sync.dma_start(
                            gi_d.ap()[m, :, jc * TC:(jc + 1) * TC, :],
                            gst[:].rearrange("p (s b) -> p s b", b=BL),
                        )

            # ---------------- Phase 2: GRU scan ----------------
            with (
                tc.tile_pool(name="gi", bufs=2) as gp,
                tc.tile_pool(name="gates", bufs=3) as ga,
                tc.tile_pool(name="ps2", bufs=4, space="PSUM") as ps2,
            ):
                gi_sb = None
                for t in range(S):
                    tci = t % TC
                    if tci == 0:
                        ch = t // TC
                        gi_sb = gp.tile([128, NM, TC, BL], F32, tag="gi")
                        for m in range(6):
                            nc.sync.dma_start(
                                gi_sb[:, m, :, :],
                                gi_d.ap()[m, :, ch * TC:(ch + 1) * TC, :],
                            )
                    if t == 0:
                        hprev = h0
                    else:
                        hprev = hseq[:, (t - 1) // 8, :, (t - 1) % 8, :]
                    gh_ps = ps2.tile([128, NM, BL], F32, tag="gh")
                    for m in range(6):
                        for k in range(2):
                            nc.tensor.matmul(
                                gh_ps[:, m, :],
                                whh_sb[:, k, m * 128:(m + 1) * 128],
                                hprev[:, k, :],
                                start=(k == 0), stop=(k == 1),
                            )
                    arz = ga.tile([128, 4, BL], F32, tag="arz")
                    nc.vector.tensor_add(arz[:], gh_ps[:, 0:4, :], gi_sb[:, 0:4, tci, :])
                    rz = ga.tile([128, 4, BL], F32, tag="rz")
                    nc.scalar.activation(rz[:], arz[:], AF.Sigmoid)
                    nh_t = ga.tile([128, 2, BL], F32, tag="nh")
                    nc.vector.tensor_add(nh_t[:], gh_ps[:, 4:6, :], bn_sb[:])
                    t1 = ga.tile([128, 2, BL], F32, tag="t1")
                    nc.vector.tensor_mul(t1[:], rz[:, 0:2, :], nh_t[:])
                    t2 = ga.tile([128, 2, BL], F32, tag="t2")
                    nc.vector.tensor_add(t2[:], t1[:], gi_sb[:, 4:6, tci, :])
                    n_t = ga.tile([128, 2, BL], F32, tag="n")
                    nc.scalar.activation(n_t[:], t2[:], AF.Tanh)
                    s_t = ga.tile([128, 2, BL], F32, tag="s")
                    nc.vector.tensor_sub(s_t[:], hprev, n_t[:])
                    sz = ga.tile([128, 2, BL], F32, tag="sz")
                    nc.vector.tensor_mul(sz[:], rz[:, 2:4, :], s_t[:])
                    nc.vector.tensor_add(hseq[:, t // 8, :, t % 8, :], n_t[:], sz[:])

            # ---------------- Phase 3: output projection ----------------
            y_re = y_d.ap().rearrange("b s v -> s b v")
            with (
                tc.tile_pool(name="yst", bufs=3) as yp,
                tc.tile_pool(name="ps3", bufs=2, space="PSUM") as ps3,
            ):
                for jb in range(S // 8):
                    y_ps = ps3.tile([128, V], F32, tag="yps")
                    for c in range(2):
                        nc.tensor.matmul(
                            y_ps[:],
                            hseq[:, jb, c, :, :],
                            wout_sb[:, c, :],
                            start=(c == 0), stop=False,
                        )
                    nc.tensor.matmul(y_ps[:], ones[0:1, :], bout_sb[0:1, :],
                                     start=False, stop=True)
                    yst = yp.tile([128, V], F32, tag="yst")
                    nc.vector.tensor_copy(yst[:], y_ps[:])
                    nc.sync.dma_start(y_re[jb * 8:(jb + 1) * 8], yst[:])

    nc.compile()
    _CACHE["nc"] = nc
    return nc


def kernel(x, embed, W_ih, b_ih, W_hh, b_hh, W_out, b_out):
    x = np.asarray(x, dtype=np.int32)
    embed = np.asarray(embed, dtype=np.float32)
    W_ih = np.asarray(W_ih, dtype=np.float32)
    b_ih = np.asarray(b_ih, dtype=np.float32)
    W_hh = np.asarray(W_hh, dtype=np.float32)
    b_hh = np.asarray(b_hh, dtype=np.float32)
    W_out = np.asarray(W_out, dtype=np.float32)
    b_out = np.asarray(b_out, dtype=np.float32)

    nc = _build()

    # r,z biases folded into the token table; n-part of b_hh applied in-scan
    bias_combo = b_ih.copy()
    bias_combo[: 2 * H] += b_hh[: 2 * H]
    a_aug = np.concatenate([embed.T, np.ones((1, V), np.float32)], axis=0)
    b_aug = np.concatenate([W_ih, bias_combo[None, :]], axis=0)
    bn = np.ascontiguousarray(
        np.broadcast_to(b_hh[2 * H:].reshape(2, 128).T[:, :, None], (128, 2, BL))
    ).reshape(128, 2 * BL)
    shared = {
        "a_aug": np.ascontiguousarray(a_aug),
        "b_aug": np.ascontiguousarray(b_aug),
        "w_hh": W_hh,
        "bn": bn,
        "w_out": W_out,
        "b_out": np.ascontiguousarray(b_out[None, :]),
    }
    in_maps = []
    for c in range(NCORES):
        xt = np.ascontiguousarray(x[c * BL:(c + 1) * BL, :].T)  # [S, BL]
        in_maps.append({"xt": xt, **shared})

    res = bass_utils.run_bass_kernel_spmd(nc, in_maps, core_ids=list(range(NCORES)))
    y = np.concatenate([res.results[c]["y"] for c in range(NCORES)], axis=0)
    return y.astype(np.float32)
